# revision 29
# baseline (speedup 1.0000x reference)
"""Trainium2 Bass kernel for causal MultiHeadAttention (B=4,S=2048,E=1024,H=16).

Sharding: 8 cores = (batch b, head-half) grid. Core c handles batch c//2 and
heads [8*(c%2), 8*(c%2)+8). Each core computes its 8 heads' attention and the
partial output projection (its 512 rows of Wo); the host sums the two partials
per batch and adds the bias (the 2-way "all-reduce" done at unshard time).

On-core dataflow (bf16 matmul operands, fp32 PSUM accumulation):
  P2: QT/KT stored head-pair-packed [64*h0 | 64*h1] on the partition dim;
      V natural [s, 8*dh] per-head tiles [V | ones | pad].
  P3: scores via 2x row-tiled matmul pairs (K=64 per head, tile_position
      rows 0/64) writing per-head PSUM tiles; exp on ACT (scale fused);
      causal via ragged diagonal-block tiles + one triangular mask multiply
      per diagonal tile (on GPSIMD). Softmax denominator = ones-column of V
      via the PV matmul's row 64.
  P4: output projection from outT [concat-head-dim, s] x Wo rows.

Scheduling: a fill-work queue (whole QK chunks, V s-tiles, lagged PV
accumulations, P4 groups) is pumped between score matmul pairs so the PE
never head-of-line blocks on the ACT-paced exp stream. ~110 tiny warmup
matmuls run during the DMA-dead prologue to hold the HAM clock gate at
K=8/8. hp3 processes chunks in order (1,2,3,0) and P4 groups chase PV(3,c)
so the tail ends on the smallest unit. Input weight DMAs dispatch from the
GPSIMD queue to keep ACT free for exp.
"""

import sys

if "/opt/trn_rl_repo" not in sys.path:
    sys.path.insert(0, "/opt/trn_rl_repo")

import numpy as np
from contextlib import ExitStack

B, S, E, H = 4, 2048, 1024, 16
DH = E // H          # 64
NCORES = 8
NH = 8               # local heads per core
HP = NH // 2         # head pairs
P = 128
NE = E // P          # 8 e-tiles
NT = S // P          # 16 s/t tiles
CH = 512
NCH = S // CH        # 4 q-chunks
SCALE = 1.0 / 8.0    # 1/sqrt(DH)

_CACHE = {}


def _build_nc():
    import concourse.mybir as mybir
    import concourse.tile as tile
    import concourse.bass as bass
    from concourse import bacc

    f32 = mybir.dt.float32
    bf16 = mybir.dt.bfloat16
    Exp = mybir.ActivationFunctionType.Exp
    PSUM = bass.MemorySpace.PSUM

    nc = bacc.Bacc(None)
    # x pre-transposed, split into two chunk-pair halves: [cp, e, 1024]
    x_d = nc.dram_tensor("x", [2, E, S // 2], bf16, kind="ExternalInput")
    # wq/wk: [p, hp, et, m] = W[et*128+p, hp*128+m]
    wq_d = nc.dram_tensor("wq", [P, HP, NE, P], bf16, kind="ExternalInput")
    wk_d = nc.dram_tensor("wk", [P, HP, NE, P], bf16, kind="ExternalInput")
    # wv: [p, et, n] = Wv[et*128+p, n]
    wv_d = nc.dram_tensor("wv", [P, NE, NH * DH], bf16, kind="ExternalInput")
    # wo: [p, ech, hp, c] = Wo[hp*128+p, ech*512+c]
    wo_d = nc.dram_tensor("wo", [P, E // CH, HP, CH], bf16, kind="ExternalInput")
    tri_d = nc.dram_tensor("tri", [P, P], bf16, kind="ExternalInput")
    out_d = nc.dram_tensor("out", [S, E], f32, kind="ExternalOutput")

    with ExitStack() as ctx:
        tc = ctx.enter_context(tile.TileContext(nc))
        persist = ctx.enter_context(tc.tile_pool(name="persist", bufs=1))
        # head-pair-packed: rows 0:64 = even head, 64:128 = odd head
        qt = persist.tile([P, HP, S], bf16)
        kt = persist.tile([P, HP, S], bf16)
        VW = 72                                       # V | ones(at 64) | pad
        vf = persist.tile([P, NT, NH, VW], bf16)
        tri = persist.tile([P, P], bf16)
        wu = persist.tile([P, 256], bf16)             # warmup weights/rhs

        wqk = ctx.enter_context(tc.tile_pool(name="wqk", bufs=1))
        otp = ctx.enter_context(tc.tile_pool(name="otp", bufs=1))
        ptp = ctx.enter_context(tc.tile_pool(name="ptp", bufs=20))
        pvo = ctx.enter_context(tc.tile_pool(name="pvo", bufs=6))
        dnp = ctx.enter_context(tc.tile_pool(name="dnp", bufs=2))
        bcp = ctx.enter_context(tc.tile_pool(name="bcp", bufs=3))
        osb = ctx.enter_context(tc.tile_pool(name="osb", bufs=2))
        drp = ctx.enter_context(tc.tile_pool(name="drp", bufs=3, space="DRAM"))
        xtp = ctx.enter_context(tc.tile_pool(name="xtp", bufs=1))
        wvp = ctx.enter_context(tc.tile_pool(name="wvp", bufs=1))
        # PSUM: sp 3x2 banks + 2 shared proj/PV banks = 8 exactly
        psA = ctx.enter_context(tc.tile_pool(name="psA", bufs=3, space=PSUM))
        psB = ctx.enter_context(tc.tile_pool(name="psB", bufs=2, space=PSUM))

        # ---- warmup: keep the PE HAM clock gate busy through the DMA-dead
        # prologue (~11us). No input dependency (memset source).
        nc.vector.memset(wu, 0.0)

        # full-array warmup matmuls: hold HAM at K=8/8 through the DMA-dead
        # prologue without tile-mode switches
        def warm_batch(n):
            for wi_ in range(n):
                pw = psA.tile([P, 2 * CH], f32, tag="sp", name="sp")
                nc.tensor.matmul(pw[:, 0:256], wu[:, 0:P], wu,
                                 start=True, stop=True)
        warm_batch(60)

        nc.sync.dma_start(out=tri, in_=tri_d[:])
        nc.vector.memset(vf.rearrange("p a b c -> p (a b c)"), 0.0)
        nc.vector.memset(vf[:, :, :, DH:DH + 1], 1.0)

        # ---- input DMA: x halves on the SYNC queue; weights on the GPSIMD
        # queue (keeps ACT free for exp).
        xts = [xtp.tile([P, S], bf16, tag=f"xt{et}", name="xt")
               for et in range(NE)]
        HS = S // 2
        for et in range(NE):
            nc.sync.dma_start(out=xts[et][:, 0:HS],
                              in_=x_d[0, et * P:(et + 1) * P, :])
        wv = wvp.tile([P, NE, NH * DH], bf16)
        nc.sync.dma_start(out=wv, in_=wv_d[:])
        for et in range(NE):
            nc.sync.dma_start(out=xts[et][:, HS:S],
                              in_=x_d[1, et * P:(et + 1) * P, :])

        wts = {}
        for hp in range(HP):
            for wi, wd in enumerate((wq_d, wk_d)):
                wt = wqk.tile([P, NE, P], bf16, tag=f"wt{hp}{wi}", name="wt")
                nc.gpsimd.dma_start(out=wt, in_=wd[:, hp])
                wts[(hp, wi)] = wt

        wt2s = []
        for ech in range(E // CH):
            wt2 = otp.tile([P, HP, CH], bf16, tag=f"wt2{ech}", name="wt2")
            nc.gpsimd.dma_start(out=wt2, in_=wo_d[:, ech])
            wt2s.append(wt2)

        outTs = [otp.tile([P, S], bf16, tag=f"outT{i}", name="outT")
                 for i in range(HP)]

        # ---- P2 emission helpers ----
        def emit_qk_chunk(hp, chk, et_outer=False):
            cs = slice(chk * CH, (chk + 1) * CH)
            if et_outer:
                # prologue form: start as soon as the first x tile lands;
                # warmup matmuls bridge the x-tile inter-arrival gaps
                pss = {}
                for wi in range(2):
                    pss[wi] = psB.tile([P, CH], f32, tag="w", name="w")
                for et in range(NE):
                    for wi in range(2):
                        nc.tensor.matmul(
                            pss[wi], wts[(hp, wi)][:, et, :], xts[et][:, cs],
                            start=(et == 0), stop=(et == NE - 1))
                    warm_batch(3)
                for wi, dst in ((0, qt), (1, kt)):
                    nc.vector.tensor_copy(out=dst[:, hp, cs], in_=pss[wi])
            else:
                for wi, dst in ((0, qt), (1, kt)):
                    ps = psB.tile([P, CH], f32, tag="w", name="w")
                    for et in range(NE):
                        nc.tensor.matmul(
                            ps, wts[(hp, wi)][:, et, :], xts[et][:, cs],
                            start=(et == 0), stop=(et == NE - 1))
                    nc.vector.tensor_copy(out=dst[:, hp, cs], in_=ps)

        def emit_v_st(st):
            """V natural for one s-tile, all 8 heads."""
            ps = psB.tile([P, NH * DH], f32, tag="w", name="w")
            for et in range(NE):
                nc.tensor.matmul(
                    ps, xts[et][:, st * P:(st + 1) * P], wv[:, et, :],
                    start=(et == 0), stop=(et == NE - 1))
            nc.vector.tensor_copy(
                out=vf[:, st, :, 0:DH],
                in_=ps.rearrange("p (h d) -> p h d", h=NH))

        # ---- P3: attention units (generator: yields after each pr) ----
        # One psA tile [128, 2*CH] holds ONE t-tile for BOTH heads (h0 in
        # cols 0:CH = bank0, h1 in CH:2CH = bank1). The two row-tiled score
        # matmuls of a (tt) write different banks and run concurrently; ONE
        # exp (2-range AP) covers both heads so both release together.
        def emit_unit(hp, chk):
            ntv = 4 * chk + 4
            nprs = ntv // 2
            pts = []          # per tt: [128, 2*CH] bf16, head h at h*CH
            for pr in range(nprs):
                tiles = []
                for j in range(2):
                    tt = 2 * pr + j
                    r = tt - 4 * chk
                    qlo = 128 * r if r > 0 else 0
                    sp = psA.tile([P, 2 * CH], f32, tag="sp", name="sp")
                    tiles.append((sp, tt, qlo))
                    for h in range(2):
                        nc.tensor.matmul(
                            sp[:, h * CH + qlo:(h + 1) * CH],
                            kt[h * DH:(h + 1) * DH, hp, tt * P:(tt + 1) * P],
                            qt[h * DH:(h + 1) * DH, hp,
                               chk * CH + qlo:(chk + 1) * CH],
                            start=True, stop=True,
                            tile_position=(h * DH, 0))
                for sp, tt, qlo in tiles:
                    pt = ptp.tile([P, 2 * CH], bf16, tag="pt", name="pt")
                    sp3 = sp.rearrange("p (h c) -> p h c", h=2)
                    pt3 = pt.rearrange("p (h c) -> p h c", h=2)
                    nc.scalar.activation(
                        out=pt3[:, :, qlo:CH], in_=sp3[:, :, qlo:CH],
                        func=Exp, scale=SCALE)
                    r = tt - 4 * chk
                    if r >= 0:
                        # triangular mask on the diagonal 128-block, per head
                        for h in range(2):
                            ms = slice(h * CH + 128 * r,
                                       h * CH + 128 * r + P)
                            nc.vector.tensor_mul(pt[:, ms], pt[:, ms], tri)
                    pts.append(pt)
                yield pts

        def emit_pv_head(hp, chk, pts, h, po, dd2):
            """PV accumulation for one head of a completed unit."""
            ntv = 4 * chk + 4
            pv = psB.tile([P, CH], f32, tag="w", name="w")
            for tt in range(ntv):
                r = tt - 4 * chk
                qlo = 128 * r if r > 0 else 0
                nc.tensor.matmul(
                    pv[0:VW, qlo:CH],
                    vf[:, tt, 2 * hp + h, :],
                    pts[tt][:, h * CH + qlo:(h + 1) * CH],
                    start=(tt == 0), stop=(tt == ntv - 1),
                    skip_group_check=True)
            nc.vector.tensor_copy(
                out=po[h * DH:(h + 1) * DH, :], in_=pv[0:DH, :])
            den = dnp.tile([1, CH], f32, tag="den", name="den")
            nc.vector.tensor_copy(out=den, in_=pv[DH:DH + 1, :])
            rdn = dnp.tile([1, CH], f32, tag="rdn", name="rdn")
            nc.vector.reciprocal_approx_fast(out=rdn, in_=den)
            nc.gpsimd.dma_start(out=dd2[h:h + 1, :], in_=rdn)

        def emit_pv_norm(hp, chk, po, dd2):
            bc = bcp.tile([P, CH], f32, tag="bc", name="bc")
            for h in range(2):
                row = dd2[h:h + 1, :]
                src = bass.AP(
                    tensor=row.tensor, offset=row.offset,
                    ap=[[0, DH]] + list(row.ap[1:]))
                nc.gpsimd.dma_start(
                    out=bc[h * DH:(h + 1) * DH, :], in_=src)
            cs = slice(chk * CH, (chk + 1) * CH)
            nc.gpsimd.tensor_mul(outTs[hp][:, cs], po, bc)

        # ---- P4: output projection (partial: local 512 rows of Wo) ----
        def emit_p4_k(ech, st4, k, ob, use_act):
            st = st4 * 4 + k
            ps = psB.tile([P, CH], f32, tag="w", name="w")
            for hp in range(HP):
                nc.tensor.matmul(
                    ps, outTs[hp][:, st * P:(st + 1) * P],
                    wt2s[ech][:, hp, :],
                    start=(hp == 0), stop=(hp == HP - 1))
            # ACT helps evacuate only in the tail (when exp is done)
            if use_act and k % 2 == 0:
                nc.scalar.copy(out=ob[:, k, :], in_=ps)
            else:
                nc.vector.tensor_copy(out=ob[:, k, :], in_=ps)
            if k % 2 == 1:
                # output DMA per 2 s-tiles, alternating queues so the last
                # groups' stores drain in parallel
                half = k // 2
                dst = out_d[(st4 * 4 + half * 2) * P:
                            (st4 * 4 + half * 2 + 2) * P,
                            ech * CH:(ech + 1) * CH]
                srcap = bass.AP(
                    tensor=dst.tensor, offset=dst.offset,
                    ap=[[dst.ap[0][0], P], [P * dst.ap[0][0], 2],
                        list(dst.ap[1])])
                eng = nc.sync if half == 0 else (
                    nc.scalar if use_act else nc.gpsimd)
                eng.dma_start(out=srcap, in_=ob[:, half * 2:half * 2 + 2, :])

        # ---- fill-work queue: (cost_ns, fn) items pumped between prs ----
        from collections import deque
        fill_q = deque()
        state = {"debt": 0}

        def push_qk(hp, chk):
            fill_q.append((3600, lambda: emit_qk_chunk(hp, chk), (hp, chk)))

        def push_v(grp):
            for st in range(4 * grp, 4 * grp + 4):
                fill_q.append((950, lambda st=st: emit_v_st(st), None))

        def push_pv(hp, chk, pts):
            po = pvo.tile([P, CH], bf16, tag="po", name="po")
            dd2 = drp.tile([2, CH], f32, tag="dd", name="dd")
            for h in range(2):
                fill_q.append(
                    ((4 * chk + 4) * 220 + 500,
                     lambda h=h: emit_pv_head(hp, chk, pts, h, po, dd2),
                     None))
            fill_q.append((200, lambda: emit_pv_norm(hp, chk, po, dd2), None))

        def push_p4(st4, use_act=False):
            for ech in range(E // CH):
                ob = osb.tile([P, 4, CH], f32, tag="ob", name="ob")
                for k in range(4):
                    fill_q.append(
                        (1150,
                         lambda ech=ech, k=k, ob=ob:
                         emit_p4_k(ech, st4, k, ob, use_act), None))

        def pump(ns):
            state["debt"] += ns
            while fill_q and state["debt"] > 0:
                cost, fn, _key = fill_q.popleft()
                fn()
                state["debt"] -= cost
            if not fill_q:
                state["debt"] = min(state["debt"], 0)

        def force_qk(key):
            """Emit any still-queued QK chunk for `key` now: its consumer
            unit is about to emit and program order defines RAW deps."""
            for item in [it for it in fill_q if it[2] == key]:
                fill_q.remove(item)
                item[1]()

        # ---- emission schedule ----
        # prologue: chunks 0 and 1 of head-pair 0, paced by the first-half
        # x DMA stream (et-outer: starts on the first landed tile)
        emit_qk_chunk(0, 0, et_outer=True)
        emit_qk_chunk(0, 1, et_outer=True)

        pend_q = deque()
        p4_chase = deque()
        qk_queue = deque((h2, c2) for h2 in range(1, HP) for c2 in range(NCH))
        hp_chunks = {0: [0, 1, 2, 3], 1: [0, 1, 2, 3],
                     2: [0, 1, 2, 3], 3: [1, 2, 3, 0]}
        ui = 0
        for hp in range(HP):
            for chk in hp_chunks[hp]:
                if hp == 0:
                    push_v(chk)              # V s-tiles for everyone
                    if 1 <= chk < NCH - 1:
                        push_qk(0, chk + 1)  # own remaining chunks (delayed
                        # one unit so they never wait on second-half x DMA)
                if ui >= 2 and qk_queue:
                    push_qk(*qk_queue.popleft())
                force_qk((hp, chk))
                gen = emit_unit(hp, chk)
                pts = None
                for pts in gen:
                    pump(2400)
                pend_q.append((hp, chk, pts))
                lag = 1 if hp == HP - 1 else 2
                while len(pend_q) > lag:
                    h0, c0, p0 = pend_q.popleft()
                    push_pv(h0, c0, p0)
                    # chase P4 one hp3-chunk behind its PV so the 5-engine
                    # normalize chain latency is hidden by the next PV
                    if h0 == 3 and p4_chase:
                        push_p4(p4_chase.popleft())
                    if h0 == 3:
                        p4_chase.append(c0)
                ui += 1
        # flush: remaining PVs (P4 groups chase one chunk behind)
        while pend_q:
            h0, c0, p0 = pend_q.popleft()
            push_pv(h0, c0, p0)
            if h0 == 3 and p4_chase:
                push_p4(p4_chase.popleft(), use_act=not pend_q)
            if h0 == 3:
                p4_chase.append(c0)
        while p4_chase:
            push_p4(p4_chase.popleft(), use_act=True)
        while fill_q:
            cost, fn, _key = fill_q.popleft()
            fn()

    nc.finalize()
    return nc


def _get_nc():
    if "nc" not in _CACHE:
        _CACHE["nc"] = _build_nc()
    return _CACHE["nc"]


def _make_in_maps(x, Wq, Wk, Wv, Wo):
    import ml_dtypes

    bf = ml_dtypes.bfloat16
    pcol = np.arange(P)[:, None]
    frow = np.arange(P)[None, :]
    tri = (pcol <= frow).astype(bf)
    in_maps = []
    for c in range(NCORES):
        b, half = divmod(c, 2)
        hs = slice(half * NH, (half + 1) * NH)
        wq = Wq[hs].transpose(1, 0, 2).reshape(E, NH * DH)
        wk = Wk[hs].transpose(1, 0, 2).reshape(E, NH * DH)
        wv = Wv[hs].transpose(1, 0, 2).reshape(E, NH * DH)
        wo = Wo[half * NH * DH:(half + 1) * NH * DH]  # [512, E]
        in_maps.append({
            "x": np.ascontiguousarray(
                x[b].T.reshape(E, 2, S // 2).transpose(1, 0, 2).astype(bf)),
            "wq": np.ascontiguousarray(
                wq.reshape(NE, P, HP, P).transpose(1, 2, 0, 3).astype(bf)),
            "wk": np.ascontiguousarray(
                wk.reshape(NE, P, HP, P).transpose(1, 2, 0, 3).astype(bf)),
            "wv": np.ascontiguousarray(
                wv.reshape(NE, P, NH * DH).transpose(1, 0, 2).astype(bf)),
            "wo": np.ascontiguousarray(
                wo.reshape(HP, P, E // CH, CH).transpose(1, 2, 0, 3)
                .astype(bf)),
            "tri": tri,
        })
    return in_maps


def _ensure_ntff_hook():
    """Register the axon NTFF profile hook under antenv.axon_hooks."""
    import types
    try:
        import antenv.axon_hooks  # noqa: F401
        return
    except ImportError:
        pass
    try:
        from trn_agent_boot.trn_boot import _ntff_profile_via_ctypes
        hook = _ntff_profile_via_ctypes("/opt/axon/libaxon_pjrt.so")
    except Exception:
        hook = None
    mod = types.ModuleType("antenv.axon_hooks")
    mod.get_axon_ntff_profile_hook = lambda: hook
    mod.set_axon_ntff_profile_hook = lambda h: None
    sys.modules["antenv.axon_hooks"] = mod


def _run(inputs, trace=False):
    from concourse.bass_utils import run_bass_kernel_spmd

    if trace:
        _ensure_ntff_hook()

    x = np.asarray(inputs["x"], dtype=np.float32)
    Wq = np.asarray(inputs["Wq"], dtype=np.float32)
    Wk = np.asarray(inputs["Wk"], dtype=np.float32)
    Wv = np.asarray(inputs["Wv"], dtype=np.float32)
    Wo = np.asarray(inputs["Wo"], dtype=np.float32)
    bo = np.asarray(inputs["bo"], dtype=np.float32)

    nc = _get_nc()
    in_maps = _make_in_maps(x, Wq, Wk, Wv, Wo)
    res = run_bass_kernel_spmd(nc, in_maps, list(range(NCORES)), trace=trace)
    out = np.empty((B, S, E), dtype=np.float32)
    for b in range(B):
        out[b] = res.results[2 * b]["out"] + res.results[2 * b + 1]["out"] + bo
    return out, res


def kernel(**inputs):
    out, _ = _run(inputs, trace=False)
    return out
